# revision 13
# baseline (speedup 1.0000x reference)
"""Trainium2 Bass kernel for nn_Demolition_splitweight_Conv2d.

Computation (per batch element b, one NeuronCore each):
    out[o, p] = (1/(127*Q)) * sum_k wvec[k] * sum_c round(Q*(conv3x3(x[c]; w[k,c,o]) + b[k,c,o]))
with Q = 12.5, wvec = [-128, 1, 2, 4, 8, 16, 32, 64].

The per-(k,c) round-to-nearest happens INSIDE the TensorEngine matmul
accumulation via the fp32 magic-number trick. The PE accumulates partial
sums sequentially within 16-row contraction sections (and combines
sections exactly in fp32-integer range), so each channel gets a 16-row
band aligned to a section:

    rows 0-8 : fp16 x taps (pre-shifted padded image rows)
    row  9   : bias_hi/1024   (rhs row holds 1024.0)
    row 10   : bias_lo/1024   (rhs 1024.0)
    row 11   : +12288         (rhs 1024.0 -> product +1.5*2^23 = magic)
    row 12   : -12288         (rhs 1024.0)
    rows 13-15: zero weights  (rhs 1024.0)

fp16 (10-bit mantissa) makes the Dekker splits of the bf16 design
unnecessary: one product row per tap. 32 bands = 512 rows = 4 chained
128-row matmuls per output half; + 2 gmat matmuls for the wvec k-sum
=> 10 matmuls per pixel block (vs 18 for the bf16 3-split layout).

Layout: data-parallel over batch (8 cores). REP [128, 4*PSZA] fp16:
partition p = 16*band + row, chunk cc (free blocks of PSZA) holds
channels ch = 8*cc + band. Only the 9 tap rows per band are uploaded
from HBM (2.4 MB); the constant-1024 rows are memset on GpSimd per
column window, pipelined ahead of the PE.
"""

import numpy as np
import ml_dtypes

import concourse.bass as bass
import concourse.mybir as mybir
from concourse.ap import AP
from concourse.tile import TileContext
from concourse.bass_utils import run_bass_kernel_spmd

# problem dims (hardcoded per the task contract)
B, C, OUT, H, W = 8, 32, 32, 64, 64
KBITS = 8
Q = 12.5
WVEC = np.array([-128, 1, 2, 4, 8, 16, 32, 64], np.float32)
SCALE = float(1.0 / (127.0 * Q))
MAG_LHS = 12288.0          # * 1024 (rhs) = 1.5*2^23
ONESV = 1024.0

PW = 66                    # padded width  (1 + 64 + 1)
PH = 66                    # padded height
PSZ = PH * PW              # 4356
USED = 4224                # max window col (64 rows of 66) (+ row 63 window)
PSZA = 4232                # chunk pitch in REP free dim
NCH = 4                    # chunks; chunk cc holds channels 8*cc..8*cc+7
NROW = 7                   # image rows per pixel block
NPIX = NROW * PW           # 462 = window pitch in REP cols
NOUT = NROW * W            # 448 = matmul moving free dim (pad cols skipped)
NPB = 10                   # pixel blocks: 9 of 7 rows + 1 of 1 row
NR_PB = [NROW] * 9 + [1]

# upload column windows: one per pixel block
WINS = [(i * NPIX, min((i + 1) * NPIX, USED)) for i in range(NPB)]

F16 = mybir.dt.float16
BF = mybir.dt.bfloat16
F32 = mybir.dt.float32

_cache = {}


def _prep_weights(weight, bias):
    """wconst16 [128, 8*128] fp16 and gmat [128, 64] bf16 host arrays."""
    w16 = np.asarray(Q * weight.astype(np.float32), np.float32).reshape(KBITS, C, OUT, 9)
    w16 = w16.astype(np.float16)
    qb = (Q * bias.astype(np.float32)).reshape(KBITS, C, OUT)
    bh = (qb / ONESV).astype(np.float16)
    bl = ((qb - bh.astype(np.float32) * ONESV) / ONESV).astype(np.float16)

    k_of = np.arange(128) // 16          # lhsT column j = k*16 + ol
    ol_of = np.arange(128) % 16
    wc = np.zeros((128, 8, 128), np.float16)   # [partition, m*4+cc, col]
    for m in range(2):
        o_of = m * 16 + ol_of
        for cc in range(NCH):
            mat = np.zeros((128, 128), np.float16)
            for band in range(8):
                ch = 8 * cc + band
                r0 = 16 * band
                mat[r0 : r0 + 9, :] = w16[k_of, ch, o_of].T
                mat[r0 + 9, :] = bh[k_of, ch, o_of]
                mat[r0 + 10, :] = bl[k_of, ch, o_of]
                mat[r0 + 11, :] = MAG_LHS
                mat[r0 + 12, :] = -MAG_LHS
            wc[:, m * 4 + cc, :] = mat
    wc = wc.reshape(128, 8 * 128)

    gmat = np.zeros((128, 64), np.float32)
    j = np.arange(128)
    gmat[j, ol_of] = WVEC[k_of]              # m0 -> out cols 0..15
    gmat[j, 32 + 16 + ol_of] = WVEC[k_of]    # m1 -> out cols 16..31
    return wc, gmat.astype(ml_dtypes.bfloat16)


def _build_xrep(x):
    """Host REP: [B, 8 band, 16 row, 4 chunk, USED] fp16.

    band b, chunk cc -> channel ch = 8*cc + b; rows 0-8 are tap-shifted
    padded images (tap t shift (t//3, t%3)), rows 9-15 hold 1024.0 for
    the bias/magic products.
    """
    x16 = np.asarray(x, np.float32).astype(np.float16)
    padw = PSZ + PW * 2 + 8
    p = np.zeros((B, C, padw), np.float16)
    p[:, :, :PSZ].reshape(B, C, PH, PW)[:, :, 1 : H + 1, 1 : W + 1] = x16
    offs = [(t // 3) * PW + (t % 3) for t in range(9)]
    sh = np.stack([p[:, :, o : o + USED] for o in offs], axis=2)  # [B, C, 9, USED]
    xrep = np.full((B, 8, 16, NCH, USED), np.float16(ONESV), np.float16)
    # taps: [B, cc, band, 9, USED] -> [B, band, 9, cc, USED]
    xrep[:, :, 0:9, :, :] = sh.reshape(B, NCH, 8, 9, USED).transpose(0, 2, 3, 1, 4)
    return np.ascontiguousarray(xrep)


def _split_multiwaits(nc):
    """This container's walrus allows one sync-wait per instruction; move
    extras onto preceding same-engine NoOps."""
    for bb in nc.main_func.blocks:
        insts = bb.instructions
        i = 0
        while i < len(insts):
            ins = insts[i]
            si = getattr(ins, "sync_info", None)
            if si is not None and si.on_wait is not None and len(si.on_wait) > 1:
                waits = list(si.on_wait)
                nops = []
                for j, w in enumerate(waits[:-1]):
                    nop = mybir.InstNoOp(name=f"{ins.name}-wsplit{j}", ins=[], outs=[])
                    nop.engine = ins.engine
                    nop.sync_info = mybir.SyncInfo(on_wait=[w], on_update=[])
                    nops.append(nop)
                si.on_wait = [waits[-1]]
                ins.sync_info = si
                for j, nop in enumerate(nops):
                    insts.insert(i + j, nop)
                i += len(nops)
            i += 1


def _build_nc():
    nc = bass.Bass()
    xrep_d = nc.dram_tensor("xrep", [128 * NCH, USED], F16, kind="ExternalInput")
    wc_d = nc.dram_tensor("wc16", [128, 8 * 128], F16, kind="ExternalInput")
    gm_d = nc.dram_tensor("gmat", [128, 64], BF, kind="ExternalInput")
    out_d = nc.dram_tensor("out", [OUT, H * W], F32, kind="ExternalOutput")

    with TileContext(nc) as tc:
        with (
            tc.tile_pool(name="const", bufs=1) as cpool,
            tc.tile_pool(name="work", bufs=6) as wpool,
            tc.tile_pool(name="outp", bufs=4) as opool,
            tc.tile_pool(name="psP", bufs=5, space="PSUM") as psP,
            tc.tile_pool(name="psR", bufs=3, space="PSUM") as psR,
        ):
            wc16 = cpool.tile([128, 8 * 128], F16, tag="wc16")
            gmat = cpool.tile([128, 64], BF, tag="gmat")
            rep = cpool.tile([128, NCH * PSZA], F16, tag="rep")
            RPITCH = NCH * PSZA

            # pstate anchor: PE busy from ~0.3us so the 3us ramp expires
            # before the first real matmul
            warm = cpool.tile([128, 64], F16, tag="warm")
            nc.gpsimd.memset(warm[:, :], 0.25)
            warm_ps = psR.tile([64, 64], F32, tag="R", name="warmps")
            nc.tensor.matmul(warm_ps[:, :], warm[:, 0:64], warm[:, :],
                             start=True, stop=True)

            # m0 lhsT first: the first conv matmuls need only cols 0..511
            nc.sync.dma_start(out=wc16[:, 0:512], in_=wc_d[:, 0:512])

            def win_dma(s0, s1, cc0=0, cc1=NCH):
                dst = AP(tensor=rep.tensor, offset=rep.offset + cc0 * PSZA + s0,
                         ap=[[RPITCH, 128], [PSZA, cc1 - cc0], [1, s1 - s0]])
                src = AP(tensor=xrep_d, offset=cc0 * USED + s0,
                         ap=[[NCH * USED, 128], [USED, cc1 - cc0], [1, s1 - s0]])
                nc.sync.dma_start(out=dst, in_=src)

            # window 0 per chunk so the m0 chain starts as early as possible
            for cc in range(NCH):
                win_dma(WINS[0][0], WINS[0][1], cc, cc + 1)
            nc.sync.dma_start(out=wc16[:, 512:1024], in_=wc_d[:, 512:1024])
            win_dma(*WINS[1])
            nc.sync.dma_start(out=gmat[:, :], in_=gm_d[:, :])
            for s0, s1 in WINS[2:]:
                win_dma(s0, s1)

            osb_shared = None
            for pb in range(NPB):
                base = pb * NPIX
                nr = NR_PB[pb]
                n = nr * W
                P01 = [psP.tile([128, NPIX], F32, tag="P", name=f"P{pb}_{i}")
                       for i in range(2)]
                P01m_win = P01
                a_tiles = []
                # m-outer: P0's chain closes early so its copy overlaps P1's
                nwin = nr * PW
                for m in range(2):
                    for cc in range(NCH):
                        rhs = rep[:, cc * PSZA + base : cc * PSZA + base + nwin]
                        w_ap = wc16[:, (m * 4 + cc) * 128 : (m * 4 + cc + 1) * 128]
                        nc.tensor.matmul(P01m_win[m][:, :nwin], w_ap, rhs,
                                         start=(cc == 0), stop=(cc == NCH - 1))
                    A = wpool.tile([128, NPIX], BF, tag="A")
                    if m == 0:
                        nc.vector.tensor_copy(A[:, :nwin], P01[m][:, :nwin])
                    else:
                        nc.scalar.copy(A[:, :nwin], P01[m][:, :nwin])
                    a_tiles.append(A)
                R = psR.tile([32, NPIX], F32, tag="R")
                nc.tensor.matmul(R[:, :nwin], gmat[:, 0:32], a_tiles[0][:, :nwin],
                                 start=True, stop=False)
                nc.tensor.matmul(R[:, :nwin], gmat[:, 32:64], a_tiles[1][:, :nwin],
                                 start=False, stop=True)

                rsrc = AP(tensor=R.tensor, offset=R.offset,
                          ap=[[NPIX, 32], [PW, nr], [1, W]])
                if pb < NPB - 2:
                    osb = opool.tile([32, NOUT], F32, tag="osb")
                    nc.vector.tensor_scalar_mul(osb[:, :n], rsrc, SCALE)
                    dst = AP(tensor=out_d, offset=pb * NOUT,
                             ap=[[H * W, OUT], [1, n]])
                    s = AP(tensor=osb.tensor, offset=osb.offset,
                           ap=[[NOUT, 32], [1, n]])
                    nc.gpsimd.dma_start(out=dst, in_=s)
                elif pb == NPB - 2:
                    # blocks 8+9 share one osb and one trailing DMA
                    osb_shared = opool.tile([32, NOUT + W], F32, tag="osb",
                                            name="osb_tail")
                    nc.vector.tensor_scalar_mul(osb_shared[:, :n], rsrc, SCALE)
                else:
                    nc.vector.tensor_scalar_mul(
                        osb_shared[:, NOUT : NOUT + n], rsrc, SCALE)
                    tw = NOUT + n
                    dst = AP(tensor=out_d, offset=(NPB - 2) * NOUT,
                             ap=[[H * W, OUT], [1, tw]])
                    s = AP(tensor=osb_shared.tensor, offset=osb_shared.offset,
                           ap=[[NOUT + W, 32], [1, tw]])
                    nc.gpsimd.dma_start(out=dst, in_=s)

    _split_multiwaits(nc)
    return nc


def kernel(x, weight, bias):
    x = np.asarray(x, np.float32)
    weight = np.asarray(weight, np.float32)
    bias = np.asarray(bias, np.float32)

    xrep = _build_xrep(x).reshape(B, 128 * NCH, USED)
    wc16, gmat = _prep_weights(weight, bias)

    if "nc" not in _cache:
        _cache["nc"] = _build_nc()
    nc = _cache["nc"]

    in_maps = [{"xrep": xrep[b], "wc16": wc16, "gmat": gmat} for b in range(B)]
    res = run_bass_kernel_spmd(nc, in_maps, core_ids=list(range(B)))
    out = np.stack([r["out"] for r in res.results])
    return out.reshape(B, OUT, H, W).astype(np.float32)


# revision 14
# speedup vs baseline: 1.0190x; 1.0190x over previous
"""Trainium2 Bass kernel for nn_Demolition_splitweight_Conv2d.

Computation (per batch element b, one NeuronCore each):
    out[o, p] = (1/(127*Q)) * sum_k wvec[k] * sum_c round(Q*(conv3x3(x[c]; w[k,c,o]) + b[k,c,o]))
with Q = 12.5, wvec = [-128, 1, 2, 4, 8, 16, 32, 64].

The per-(k,c) round-to-nearest happens INSIDE the TensorEngine matmul
accumulation via the fp32 magic-number trick. The PE accumulates partial
sums sequentially within 16-row contraction sections (and combines
sections exactly in fp32-integer range), so each channel gets a 16-row
band aligned to a section:

    rows 0-8 : fp16 x taps (pre-shifted padded image rows)
    row  9   : bias_hi/1024   (rhs row holds 1024.0)
    row 10   : bias_lo/1024   (rhs 1024.0)
    row 11   : +12288         (rhs 1024.0 -> product +1.5*2^23 = magic)
    row 12   : -12288         (rhs 1024.0)
    rows 13-15: zero weights  (rhs 1024.0)

fp16 (10-bit mantissa) makes the Dekker splits of the bf16 design
unnecessary: one product row per tap. 32 bands = 512 rows = 4 chained
128-row matmuls per output half; + 2 gmat matmuls for the wvec k-sum
=> 10 matmuls per pixel block (vs 18 for the bf16 3-split layout).

Layout: data-parallel over batch (8 cores). REP [128, 4*PSZA] fp16:
partition p = 16*band + row, chunk cc (free blocks of PSZA) holds
channels ch = 8*cc + band. Only the 9 tap rows per band are uploaded
from HBM (2.4 MB); the constant-1024 rows are memset on GpSimd per
column window, pipelined ahead of the PE.
"""

import numpy as np
import ml_dtypes

import concourse.bass as bass
import concourse.mybir as mybir
from concourse.ap import AP
from concourse.tile import TileContext
from concourse.bass_utils import run_bass_kernel_spmd

# problem dims (hardcoded per the task contract)
B, C, OUT, H, W = 8, 32, 32, 64, 64
KBITS = 8
Q = 12.5
WVEC = np.array([-128, 1, 2, 4, 8, 16, 32, 64], np.float32)
SCALE = float(1.0 / (127.0 * Q))
MAG_LHS = 12288.0          # * 1024 (rhs) = 1.5*2^23
ONESV = 1024.0

PW = 66                    # padded width  (1 + 64 + 1)
PH = 66                    # padded height
PSZ = PH * PW              # 4356
USED = 4224                # max window col (64 rows of 66) (+ row 63 window)
PSZA = 4232                # chunk pitch in REP free dim
NCH = 4                    # chunks; chunk cc holds channels 8*cc..8*cc+7
NROW = 7                   # image rows per pixel block
NPIX = NROW * PW           # 462 = window pitch in REP cols
NOUT = NROW * W            # 448 = matmul moving free dim (pad cols skipped)
NPB = 10                   # pixel blocks: 9 of 7 rows + 1 of 1 row
NR_PB = [NROW] * 9 + [1]

# upload column windows: one per pixel block
WINS = [(i * NPIX, min((i + 1) * NPIX, USED)) for i in range(NPB)]

F16 = mybir.dt.float16
BF = mybir.dt.bfloat16
F32 = mybir.dt.float32

_cache = {}


def _prep_weights(weight, bias):
    """wconst16 [128, 8*128] fp16 and gmat [128, 64] bf16 host arrays."""
    w16 = np.asarray(Q * weight.astype(np.float32), np.float32).reshape(KBITS, C, OUT, 9)
    w16 = w16.astype(np.float16)
    qb = (Q * bias.astype(np.float32)).reshape(KBITS, C, OUT)
    bh = (qb / ONESV).astype(np.float16)
    bl = ((qb - bh.astype(np.float32) * ONESV) / ONESV).astype(np.float16)

    k_of = np.arange(128) // 16          # lhsT column j = k*16 + ol
    ol_of = np.arange(128) % 16
    wc = np.zeros((128, 8, 128), np.float16)   # [partition, m*4+cc, col]
    for m in range(2):
        o_of = m * 16 + ol_of
        for cc in range(NCH):
            mat = np.zeros((128, 128), np.float16)
            for band in range(8):
                ch = 8 * cc + band
                r0 = 16 * band
                mat[r0 : r0 + 9, :] = w16[k_of, ch, o_of].T
                mat[r0 + 9, :] = bh[k_of, ch, o_of]
                mat[r0 + 10, :] = bl[k_of, ch, o_of]
                mat[r0 + 11, :] = MAG_LHS
                mat[r0 + 12, :] = -MAG_LHS
            wc[:, m * 4 + cc, :] = mat
    wc = wc.reshape(128, 8 * 128)

    gmat = np.zeros((128, 64), np.float32)
    j = np.arange(128)
    gmat[j, ol_of] = WVEC[k_of]              # m0 -> out cols 0..15
    gmat[j, 32 + 16 + ol_of] = WVEC[k_of]    # m1 -> out cols 16..31
    return wc, gmat.astype(ml_dtypes.bfloat16)


def _build_xrep(x):
    """Host REP: [B, 8 band, 16 row, 4 chunk, USED] fp16.

    band b, chunk cc -> channel ch = 8*cc + b; rows 0-8 are tap-shifted
    padded images (tap t shift (t//3, t%3)), rows 9-15 hold 1024.0 for
    the bias/magic products.
    """
    x16 = np.asarray(x, np.float32).astype(np.float16)
    padw = PSZ + PW * 2 + 8
    p = np.zeros((B, C, padw), np.float16)
    p[:, :, :PSZ].reshape(B, C, PH, PW)[:, :, 1 : H + 1, 1 : W + 1] = x16
    offs = [(t // 3) * PW + (t % 3) for t in range(9)]
    sh = np.stack([p[:, :, o : o + USED] for o in offs], axis=2)  # [B, C, 9, USED]
    xrep = np.full((B, 8, 16, NCH, USED), np.float16(ONESV), np.float16)
    # taps: [B, cc, band, 9, USED] -> [B, band, 9, cc, USED]
    xrep[:, :, 0:9, :, :] = sh.reshape(B, NCH, 8, 9, USED).transpose(0, 2, 3, 1, 4)
    return np.ascontiguousarray(xrep)


def _split_multiwaits(nc):
    """This container's walrus allows one sync-wait per instruction; move
    extras onto preceding same-engine NoOps."""
    for bb in nc.main_func.blocks:
        insts = bb.instructions
        i = 0
        while i < len(insts):
            ins = insts[i]
            si = getattr(ins, "sync_info", None)
            if si is not None and si.on_wait is not None and len(si.on_wait) > 1:
                waits = list(si.on_wait)
                nops = []
                for j, w in enumerate(waits[:-1]):
                    nop = mybir.InstNoOp(name=f"{ins.name}-wsplit{j}", ins=[], outs=[])
                    nop.engine = ins.engine
                    nop.sync_info = mybir.SyncInfo(on_wait=[w], on_update=[])
                    nops.append(nop)
                si.on_wait = [waits[-1]]
                ins.sync_info = si
                for j, nop in enumerate(nops):
                    insts.insert(i + j, nop)
                i += len(nops)
            i += 1


def _build_nc():
    nc = bass.Bass()
    xrep_d = nc.dram_tensor("xrep", [128 * NCH, USED], F16, kind="ExternalInput")
    wc_d = nc.dram_tensor("wc16", [128, 8 * 128], F16, kind="ExternalInput")
    gm_d = nc.dram_tensor("gmat", [128, 64], BF, kind="ExternalInput")
    out_d = nc.dram_tensor("out", [OUT, H * W], F32, kind="ExternalOutput")

    with TileContext(nc) as tc:
        with (
            tc.tile_pool(name="const", bufs=1) as cpool,
            tc.tile_pool(name="work", bufs=6) as wpool,
            tc.tile_pool(name="outp", bufs=4) as opool,
            tc.tile_pool(name="psP", bufs=5, space="PSUM") as psP,
            tc.tile_pool(name="psR", bufs=3, space="PSUM") as psR,
        ):
            wc16 = cpool.tile([128, 8 * 128], F16, tag="wc16")
            gmat = cpool.tile([128, 64], BF, tag="gmat")
            rep = cpool.tile([128, NCH * PSZA], F16, tag="rep")
            RPITCH = NCH * PSZA

            # pstate anchor: PE busy from ~0.3us so the 3us ramp expires
            # before the first real matmul
            warm = cpool.tile([128, 64], F16, tag="warm")
            nc.gpsimd.memset(warm[:, :], 0.25)
            warm_ps = psR.tile([64, 64], F32, tag="R", name="warmps")
            nc.tensor.matmul(warm_ps[:, :], warm[:, 0:64], warm[:, :],
                             start=True, stop=True)

            # m0 lhsT first: the first conv matmuls need only cols 0..511
            nc.sync.dma_start(out=wc16[:, 0:512], in_=wc_d[:, 0:512])

            def win_dma(s0, s1, cc0=0, cc1=NCH):
                dst = AP(tensor=rep.tensor, offset=rep.offset + cc0 * PSZA + s0,
                         ap=[[RPITCH, 128], [PSZA, cc1 - cc0], [1, s1 - s0]])
                src = AP(tensor=xrep_d, offset=cc0 * USED + s0,
                         ap=[[NCH * USED, 128], [USED, cc1 - cc0], [1, s1 - s0]])
                nc.sync.dma_start(out=dst, in_=src)

            # window 0 per chunk so the m0 chain starts as early as possible
            for cc in range(NCH):
                win_dma(WINS[0][0], WINS[0][1], cc, cc + 1)
            nc.sync.dma_start(out=wc16[:, 512:1024], in_=wc_d[:, 512:1024])
            win_dma(*WINS[1])
            nc.sync.dma_start(out=gmat[:, :], in_=gm_d[:, :])
            for s0, s1 in WINS[2:]:
                win_dma(s0, s1)

            osb_shared = None
            for pb in range(NPB):
                base = pb * NPIX
                nr = NR_PB[pb]
                n = nr * W
                P01 = [psP.tile([128, NPIX], F32, tag="P", name=f"P{pb}_{i}")
                       for i in range(2)]
                P01m_win = P01
                a_tiles = []
                # m-outer: P0's chain closes early so its copy overlaps P1's
                nwin = nr * PW
                for m in range(2):
                    for cc in range(NCH):
                        rhs = rep[:, cc * PSZA + base : cc * PSZA + base + nwin]
                        w_ap = wc16[:, (m * 4 + cc) * 128 : (m * 4 + cc + 1) * 128]
                        nc.tensor.matmul(P01m_win[m][:, :nwin], w_ap, rhs,
                                         start=(cc == 0), stop=(cc == NCH - 1))
                    A = wpool.tile([128, NPIX], BF, tag="A")
                    if m == 0:
                        nc.vector.tensor_copy(A[:, :nwin], P01[m][:, :nwin])
                    else:
                        nc.scalar.copy(A[:, :nwin], P01[m][:, :nwin])
                    a_tiles.append(A)
                R = psR.tile([32, NPIX], F32, tag="R")
                nc.tensor.matmul(R[:, :nwin], gmat[:, 0:32], a_tiles[0][:, :nwin],
                                 start=True, stop=False)
                nc.tensor.matmul(R[:, :nwin], gmat[:, 32:64], a_tiles[1][:, :nwin],
                                 start=False, stop=True)

                rsrc = AP(tensor=R.tensor, offset=R.offset,
                          ap=[[NPIX, 32], [PW, nr], [1, W]])
                if pb < NPB - 2:
                    osb = opool.tile([32, NOUT], F32, tag="osb")
                    nc.vector.tensor_scalar_mul(osb[:, :n], rsrc, SCALE)
                    dst = AP(tensor=out_d, offset=pb * NOUT,
                             ap=[[H * W, OUT], [1, n]])
                    s = AP(tensor=osb.tensor, offset=osb.offset,
                           ap=[[NOUT, 32], [1, n]])
                    nc.scalar.dma_start(out=dst, in_=s)
                elif pb == NPB - 2:
                    # blocks 8+9 share one osb and one trailing DMA
                    osb_shared = opool.tile([32, NOUT + W], F32, tag="osb",
                                            name="osb_tail")
                    nc.vector.tensor_scalar_mul(osb_shared[:, :n], rsrc, SCALE)
                else:
                    nc.vector.tensor_scalar_mul(
                        osb_shared[:, NOUT : NOUT + n], rsrc, SCALE)
                    tw = NOUT + n
                    dst = AP(tensor=out_d, offset=(NPB - 2) * NOUT,
                             ap=[[H * W, OUT], [1, tw]])
                    s = AP(tensor=osb_shared.tensor, offset=osb_shared.offset,
                           ap=[[NOUT + W, 32], [1, tw]])
                    nc.scalar.dma_start(out=dst, in_=s)

    _split_multiwaits(nc)
    return nc


def kernel(x, weight, bias):
    x = np.asarray(x, np.float32)
    weight = np.asarray(weight, np.float32)
    bias = np.asarray(bias, np.float32)

    xrep = _build_xrep(x).reshape(B, 128 * NCH, USED)
    wc16, gmat = _prep_weights(weight, bias)

    if "nc" not in _cache:
        _cache["nc"] = _build_nc()
    nc = _cache["nc"]

    in_maps = [{"xrep": xrep[b], "wc16": wc16, "gmat": gmat} for b in range(B)]
    res = run_bass_kernel_spmd(nc, in_maps, core_ids=list(range(B)))
    out = np.stack([r["out"] for r in res.results])
    return out.reshape(B, OUT, H, W).astype(np.float32)
